# revision 10
# baseline (speedup 1.0000x reference)
"""Trainium2 Bass kernel for EquivariantReconstructionLayer.

reference:
    si   = |v|                      [B, 64]        (v: [B, 64, 3])
    h    = relu(si @ W1 + b1)       [B, 32]
    sw   = (h @ W2 + b2) -> [B, 128, 64]
    m    = v v^T                    [B, 64, 3, 3]
    out  = einsum('brc,bcij->brij', sw, m)   [B, 128, 3, 3]

Sharding: pure data-parallel over B across 8 cores.

Device-side design (per core, 12800 padded atoms, 25 tiles x 512 atoms):
  - Host pre-transposes v to channel-major bf16 "vt_lo"/"vt_hi"
    [tile, 128, 3, 512]: partitions (0:64)=c for lo (upper zero),
    (64:128)=c for hi (lower zero).  This is layout prep only - all
    arithmetic happens on device.
  - M6 [128, 12, N] bf16: 12 monomial columns (xx yy zz xy xz yz) in a
    block-diagonal arrangement: rows 0:64 hold cols 0:6, rows 64:128 hold
    cols 6:12 (zeros elsewhere, produced free by the zeroed halves of
    vt_lo/vt_hi).  Column m of the 3x3-symmetric outer product basis.
  - si2 = xx+yy+zz (cols 0..2 of M6), siT = sqrt (ACT).
  - W1 matmul col-tiled 4x -> h PSUM [128, N] = 4 copies of [32, N];
    ACT relu+bias -> hT4 bf16 (quadded so 4-way row-tiled W2 matmuls can
    stream their own partition group).
  - W2 as 64 "pair" matmuls: lhsT [32, 128] = W2 columns for r=2j,2j+1
    (64 channels each), 4-way row-tiled.  PSUM [128=(parity,c), N] ->
    bf16 copy into SW [128, 64, N].
  - Contraction: per atom, matmul(lhsT=M6[:, :, n] (block-diag 12 cols),
    rhs=SW[:, :, n] [128, 64 r-pairs]) -> PSUM [12, 64], 4 atoms packed
    via col-tiling at 32-partition pitch, 8 atoms along PSUM free dim.
  - Stage [128, 8, 64] f32 -> DMA out as out[g, a, 12, 8, 64].
  - Host expands the 6 unique symmetric entries to the 9 (i,j) entries
    and permutes to [B, 128, 3, 3].
"""

import numpy as np
import ml_dtypes

BF16 = ml_dtypes.bfloat16

B, C, R = 100000, 64, 128
NCORES = 8
BPC = 12800               # padded atoms per core
NT = 512                  # atoms per tile
TILES = BPC // NT         # 25
NPAIR = R // 2            # 64 r-pairs
GROUPS = TILES * 16       # 400 32-atom output groups per core

_g = {"nc": None}


def _install_wait_legalizer():
    """The walrus codegen used by the bass2jax/PJRT path supports at most ONE
    sync-wait per instruction ("Too many sync wait commands").  Tile inlines
    multi-waits freely (the native bass execution path handles them).  Fix:
    post-process the BIR JSON, hoisting all but one wait of each instruction
    into standalone EventSemaphore (wait-only nop) instructions on the same
    engine queue immediately before it."""
    import json
    from concourse import bass

    if getattr(bass.Bass, "_wait_legalizer_installed", False):
        return
    orig = bass.Bass.to_json_bytes

    def patched(self):
        raw = orig(self)
        j = json.loads(raw)
        cnt = 0
        for fn in j.get("functions", []):
            for blk in fn.get("blocks", []):
                insts = blk.get("instructions", [])
                out = []
                for ins in insts:
                    si = ins.get("sync_info") or {}
                    ow = si.get("on_wait") or []
                    if len(ow) > 1:
                        for w in ow[:-1]:
                            cnt += 1
                            out.append({
                                "debug": ins.get("debug", 0),
                                "engine": ins["engine"],
                                "ins": [],
                                "name": f"legalw_{cnt}",
                                "opcode": "EventSemaphore",
                                "outs": [],
                                "sync_info": {"on_update": [], "on_wait": [w]},
                            })
                        si["on_wait"] = [ow[-1]]
                    out.append(ins)
                blk["instructions"] = out
        return json.dumps(j).encode()

    bass.Bass.to_json_bytes = patched
    bass.Bass._wait_legalizer_installed = True


def _build_bass(reps=1):
    from concourse import bass, tile, mybir

    _install_wait_legalizer()

    f32 = mybir.dt.float32
    bf16 = mybir.dt.bfloat16
    AF = mybir.ActivationFunctionType

    nc = bass.Bass()

    vt_lo_d = nc.declare_dram_parameter("vt_lo", [TILES, 128, 3, NT], bf16, isOutput=False)
    vt_hi_d = nc.declare_dram_parameter("vt_hi", [TILES, 128, 3, NT], bf16, isOutput=False)
    w1_d = nc.declare_dram_parameter("w1", [C, 32], bf16, isOutput=False)
    b1q_d = nc.declare_dram_parameter("b1q", [128, 1], f32, isOutput=False)
    w2s_d = nc.declare_dram_parameter("w2s", [128, 16, 128], bf16, isOutput=False)
    out_d = nc.declare_dram_parameter("out", [GROUPS, 4, 12, 8, 64], f32, isOutput=True)

    # monomial order: xx yy zz xy xz yz
    MONO = [(0, 0), (1, 1), (2, 2), (0, 1), (0, 2), (1, 2)]

    with tile.TileContext(nc) as tc:
        with (
            tc.tile_pool(name="const", bufs=1) as constp,
            tc.tile_pool(name="vt", bufs=2) as vtp,
            tc.tile_pool(name="m6", bufs=2) as m6p,
            tc.tile_pool(name="small", bufs=2) as smallp,
            tc.tile_pool(name="sw", bufs=2) as swp,
            tc.tile_pool(name="stage", bufs=3) as stagep,
            tc.tile_pool(name="hps", bufs=2, space="PSUM") as hpsp,
            tc.tile_pool(name="pps", bufs=2, space="PSUM") as ppsp,
            tc.tile_pool(name="ops", bufs=2, space="PSUM") as opsp,
        ):
            w1_t = constp.tile([C, 32], bf16)
            b1q_t = constp.tile([128, 1], f32)
            w2s_t = constp.tile([128, 16, 128], bf16)
            nc.sync.dma_start(out=w1_t[:], in_=w1_d[:])
            nc.sync.dma_start(out=b1q_t[:], in_=b1q_d[:])
            nc.sync.dma_start(out=w2s_t[:], in_=w2s_d[:])

            def emit_all():
              for t in range(TILES):
                vlo = vtp.tile([128, 3, NT], bf16, tag="vlo")
                vhi = vtp.tile([128, 3, NT], bf16, tag="vhi")
                nc.sync.dma_start(out=vlo[:], in_=vt_lo_d[t])
                nc.sync.dma_start(out=vhi[:], in_=vt_hi_d[t])

                m6 = m6p.tile([128, 12, NT], bf16)
                for m, (i, j) in enumerate(MONO):
                    # lower block: rows 0:64 data, rows 64:128 zero
                    nc.vector.tensor_mul(m6[:, m, :], vlo[:, i, :], vlo[:, j, :])
                    nc.vector.tensor_mul(m6[:, 6 + m, :], vhi[:, i, :], vhi[:, j, :])

                si2a = smallp.tile([C, NT], bf16, tag="si2a")
                si2b = smallp.tile([C, NT], bf16, tag="si2b")
                sit = smallp.tile([C, NT], bf16, tag="sit")
                nc.vector.tensor_add(si2a[:], m6[0:C, 0, :], m6[0:C, 1, :])
                nc.vector.tensor_add(si2b[:], si2a[:], m6[0:C, 2, :])
                nc.scalar.activation(sit[:], si2b[:], AF.Sqrt)

                # W1: 4 col-tiled copies -> h PSUM [128, N]
                hps = hpsp.tile([128, NT], f32)
                for jc in range(4):
                    nc.tensor.matmul(
                        hps[32 * jc : 32 * jc + 32, :],
                        w1_t[:],
                        sit[:],
                        tile_position=(0, 32 * jc),
                    )
                ht4 = smallp.tile([128, NT], bf16, tag="ht4")
                nc.scalar.activation(ht4[:], hps[:], AF.Relu, bias=b1q_t[:])

                # W2 pair matmuls -> SW
                sw = swp.tile([128, NPAIR, NT], bf16)
                for half in range(NPAIR // 2):
                    pps = ppsp.tile([128, 2, NT], f32)
                    for s in range(2):
                        jp = 2 * half + s
                        rg = jp % 4
                        nc.tensor.matmul(
                            pps[:, s, :],
                            w2s_t[32 * rg : 32 * rg + 32, jp // 4, :],
                            ht4[32 * rg : 32 * rg + 32, :],
                            tile_position=(32 * rg, 0),
                        )
                    if half % 2 == 0:
                        nc.vector.tensor_copy(sw[:, 2 * half : 2 * half + 2, :], pps[:])
                    else:
                        nc.scalar.activation(sw[:, 2 * half : 2 * half + 2, :], pps[:], AF.Copy)

                # contraction: 32 atoms per psum tile
                for q in range(16):
                    ops = opsp.tile([128, 8, 64], f32)
                    for jj in range(8):
                        for a in range(4):
                            n = q * 32 + jj * 4 + a
                            nc.tensor.matmul(
                                ops[32 * a : 32 * a + 12, jj, :],
                                m6[:, :, n],
                                sw[:, :, n],
                                tile_position=(0, 32 * a),
                            )
                    st = stagep.tile([128, 8, 64], f32)
                    if q % 2 == 0:
                        nc.vector.tensor_copy(st[:], ops[:])
                    else:
                        nc.scalar.activation(st[:], ops[:], AF.Copy)
                    g = t * 16 + q
                    for a in range(4):
                        nc.sync.dma_start(
                            out=out_d[g, a],
                            in_=st[32 * a : 32 * a + 12, :, :],
                        )

            if reps == 1:
                emit_all()
            else:
                with tc.For_i(0, reps, 1):
                    emit_all()
    return nc


def _host_prep(compressed_vectors, W1, b1, W2, b2):
    v = np.asarray(compressed_vectors, dtype=np.float32)
    vp = np.zeros((NCORES * BPC, C, 3), np.float32)
    vp[: v.shape[0]] = v
    # [core, tile, n, c, comp] -> [core, tile, c, comp, n]
    vt = vp.reshape(NCORES, TILES, NT, C, 3).transpose(0, 1, 3, 4, 2)
    vt = np.ascontiguousarray(vt).astype(BF16)
    vt_lo = np.zeros((NCORES, TILES, 128, 3, NT), BF16)
    vt_hi = np.zeros((NCORES, TILES, 128, 3, NT), BF16)
    vt_lo[:, :, 0:C] = vt
    vt_hi[:, :, C:128] = vt

    w1b = np.asarray(W1, np.float32).astype(BF16)          # [64, 32]
    b1q = np.tile(np.asarray(b1, np.float32), 4).reshape(128, 1)
    # w2s[32*i + k, g, p*64 + c] = W2[k, (2*(4g+i)+p)*64 + c]
    w2r = np.asarray(W2, np.float32).reshape(32, 16, 4, 2, C)  # [k, g, i, p, c]
    w2s = np.ascontiguousarray(w2r.transpose(2, 0, 1, 3, 4)).reshape(4, 32, 16, 128)
    w2s = np.ascontiguousarray(w2s).reshape(128, 16, 128).astype(BF16)
    return vt_lo, vt_hi, w1b, b1q.astype(np.float32), w2s


# index of monomial (xx yy zz xy xz yz) for each of the 9 (i,j) entries
_IJ_IDX = np.array([0, 3, 4, 3, 1, 5, 4, 5, 2])


def _host_post(results):
    o = np.stack([np.asarray(r["out"]) for r in results])  # [8, 400, 4, 12, 8, 64]
    o = o.reshape(NCORES, GROUPS, 4, 2, 6, 8, 64)          # [core,g,a,p,m,jj,k]
    o = o.transpose(0, 1, 5, 2, 6, 3, 4)                   # [core,g,jj,a,k,p,m]
    # atoms in order core,(t,q),jj,a ; r = 2k+p is consecutive in (k,p)
    o = np.ascontiguousarray(o).reshape(NCORES * BPC, R, 6)
    out9 = o[:, :, _IJ_IDX]                                # [atoms, 128, 9]
    return np.ascontiguousarray(out9[:B]).reshape(B, R, 3, 3).astype(np.float32)


def run_device(inputs_per_core, trace=False):
    from concourse.bass_utils import run_bass_kernel_spmd

    if _g["nc"] is None:
        _g["nc"] = _build_bass()
    res = run_bass_kernel_spmd(
        _g["nc"], inputs_per_core, core_ids=list(range(NCORES)), trace=trace
    )
    return res


def kernel(compressed_vectors, W1, b1, W2, b2, _trace=False, _return_raw=False):
    vt_lo, vt_hi, w1b, b1q, w2s = _host_prep(compressed_vectors, W1, b1, W2, b2)
    in_maps = [
        {
            "vt_lo": vt_lo[c],
            "vt_hi": vt_hi[c],
            "w1": w1b,
            "b1q": b1q,
            "w2s": w2s,
        }
        for c in range(NCORES)
    ]
    res = run_device(in_maps, trace=_trace)
    out = _host_post(res.results)
    b2a = np.asarray(b2, np.float32)
    if np.any(b2a):
        # spec fills b2 with zeros; correctness fallback for nonzero b2
        v = np.asarray(compressed_vectors, np.float32)
        m9 = (v[:, :, :, None] * v[:, :, None, :]).reshape(B, C, 9)
        corr = np.tensordot(b2a.reshape(R, C), m9, axes=([1], [1]))  # [R, B, 9]
        out = out + corr.transpose(1, 0, 2).reshape(B, R, 3, 3)
    if _return_raw:
        return out, res
    return out
